# revision 7
# baseline (speedup 1.0000x reference)
"""GCN layer (degree-normalized copy-src/sum) on 8 TRN2 NeuronCores — v2.

Design (single launch per core, SPMD over 8 cores):
  - node table bf16 [100352, 64]; viewed as [25088, 256] = 4-node groups of
    512B so SWDGE dma_gather int16 indices cover the full table (src//4)
    with no buckets and no <512B DMA latency penalty.
  - out-degree + in-degree histograms on device (sorted-residual run-length
    via GPSIMD local_scatter, f16 streams).
  - scale pass: scaled = node * rsqrt(clip(out_deg,1)) (bf16, DRAM scratch).
  - edges sharded by dst slice owner; per dst window of 128 rows, edges
    padded to 18 tiles of 128 slots; gather chunks of 1024 slots.
  - per chunk: DVE blend selects each edge's true row out of its 4-node
    group (quarter one-hot), DVE builds dst one-hot masks, PE matmuls
    aggregate into PSUM per window (f32 accumulation — no scatter at all).
  - psum * rsqrt(clip(in_deg,1)) -> output slice.
"""

import sys

sys.path.insert(0, "/opt/trn_rl_repo")

import numpy as np
import ml_dtypes


class CFG:
    N = 100000
    D = 64
    NC = 8
    SLICE = 12544          # dst rows per core
    W = 98                 # dst windows of 128 per core
    TPW = 18               # tiles (128 slots) per window
    SLOTS = 98 * 18 * 128  # 225792
    CH = 1024              # gather slots per instruction
    NCH = (SLOTS + CH - 1) // CH  # 221 (tail 512 slots unused)
    NPAD = 100352
    NG = 100352 // 4       # 4-node groups
    SC_SRC = 13312         # src hist stream cols (full graph, all cores)
    SC_DST = 2048          # dst hist stream cols (slice local)
    TW_SRC = 784           # nodes per partition (full)
    TW_DST = 98            # dst rows per partition (slice)


# ------------------------------------------------------------- host prep ---
def _wrap16(a):
    """[1024] int16 -> [128, 64] wrapped in 16 partitions, replicated."""
    w = a.reshape(64, 16).T.astype(np.int16)
    return np.tile(w, (8, 1))


def _hist_stream_pr(p, r, sc):
    """Explicit (partition, residual) pairs -> [128, sc] f16 sorted-residual
    stream padded with -1."""
    order = np.lexsort((r, p))
    po, ro = p[order], r[order]
    cnts = np.bincount(po, minlength=128)
    if cnts.max() > sc:
        raise ValueError(f"hist stream overflow: {cnts.max()} > {sc}")
    st = np.full((128, sc), -1.0, np.float16)
    starts = np.concatenate([[0], np.cumsum(cnts)[:-1]])
    pos = np.arange(len(p)) - starts[po]
    st[po, pos] = ro.astype(np.float16)
    return st


def _hist_stream(vals, tw, sc):
    """Values v in [0, 128*tw): partition v%128, residual v//128."""
    return _hist_stream_pr(vals % 128, vals // 128, sc)


def host_prep(cfg, u_f, v_f, src, dst):
    node = np.zeros((cfg.NPAD, cfg.D), np.float32)
    node[: u_f.shape[0]] = u_f
    node[u_f.shape[0] : u_f.shape[0] + v_f.shape[0]] = v_f
    # interleaved 4-node groups: node_iv[g, f*4+q] = node[4g+q, f]
    node_b = np.ascontiguousarray(
        node.astype(ml_dtypes.bfloat16)
        .reshape(cfg.NG, 4, cfg.D)
        .transpose(0, 2, 1)
        .reshape(cfg.NG, cfg.D * 4)
    )

    src = np.asarray(src, dtype=np.int64)
    dst = np.asarray(dst, dtype=np.int64)

    # src hist in interleaved layout: node n -> partition (n//4)%128,
    # residual ((n//4)//128)*4 + n%4  (so w_out[:, G*4+q] matches the
    # scale pass tiling over group rows G)
    g_all = src // 4
    shist = _hist_stream_pr(g_all % 128, (g_all // 128) * 4 + src % 4,
                            cfg.SC_SRC)

    ins = []
    for k in range(cfg.NC):
        m = (dst // cfg.SLICE) == k
        es = src[m]
        ed = dst[m] - k * cfg.SLICE
        dhist = _hist_stream(ed, cfg.TW_DST, cfg.SC_DST)

        order = np.argsort(ed, kind="stable")
        es, ed = es[order], ed[order]
        w = ed // 128
        wcnt = np.bincount(w, minlength=cfg.W)
        if wcnt.max() > cfg.TPW * 128:
            raise ValueError(f"window overflow: {wcnt.max()}")
        # slot for edge i: w*TPW*128 + rank within window
        starts = np.concatenate([[0], np.cumsum(wcnt)[:-1]])
        rank = np.arange(len(ed)) - starts[w]
        slot = w * cfg.TPW * 128 + rank

        tot = cfg.NCH * cfg.CH
        gidx = np.zeros(tot, np.int16)          # pad -> group 0
        dstv = np.full(tot, -1.0, ml_dtypes.bfloat16)
        qind = np.zeros((tot, 4), ml_dtypes.bfloat16)
        gidx[slot] = (es // 4).astype(np.int16)
        dstv[slot] = (ed % 128).astype(np.float32).astype(ml_dtypes.bfloat16)
        qind[slot, es % 4] = 1.0

        gidx_w = np.stack([_wrap16(gidx[c * cfg.CH : (c + 1) * cfg.CH])
                           for c in range(cfg.NCH)])          # [NCH,128,64]
        # slot s: chunk c = s//1024, tile k=(s%1024)//128, partition e=s%128
        dstv_w = dstv.reshape(cfg.NCH, 8, 128).transpose(0, 2, 1)
        qind_w = qind.reshape(cfg.NCH, 8, 128, 4).transpose(0, 2, 1, 3)
        ins.append({
            "node_tbl": node_b, "shist": shist, "dhist": dhist,
            "gidx": gidx_w, "dstv": np.ascontiguousarray(dstv_w),
            "qind": np.ascontiguousarray(qind_w),
        })
    return ins


# ----------------------------------------------------------- device build ---
def build(cfg):
    import concourse.tile as tile
    from concourse import bacc, mybir

    dt = mybir.dt
    alu = mybir.AluOpType

    nc = bacc.Bacc("TRN2", target_bir_lowering=False, debug=False,
                   num_devices=cfg.NC, num_swdge_queues=4)
    node_t = nc.dram_tensor("node_tbl", [cfg.NG, cfg.D * 4], dt.bfloat16,
                            kind="ExternalInput")
    shist_t = nc.dram_tensor("shist", [128, cfg.SC_SRC], dt.float16,
                             kind="ExternalInput")
    dhist_t = nc.dram_tensor("dhist", [128, cfg.SC_DST], dt.float16,
                             kind="ExternalInput")
    gidx_t = nc.dram_tensor("gidx", [cfg.NCH, 128, cfg.CH // 16], dt.int16,
                            kind="ExternalInput")
    dstv_t = nc.dram_tensor("dstv", [cfg.NCH, 128, 8], dt.bfloat16,
                            kind="ExternalInput")
    qind_t = nc.dram_tensor("qind", [cfg.NCH, 128, 8, 4], dt.bfloat16,
                            kind="ExternalInput")
    out_t = nc.dram_tensor("rst", [cfg.W, 128, cfg.D], dt.float32,
                           kind="ExternalOutput")

    with tile.TileContext(nc) as tc:
        with tc.tile_pool(name="dram", bufs=1, space="DRAM") as dpool:
            scaled = dpool.tile([cfg.NG, cfg.D * 4], dt.bfloat16)

            # ---------------- histograms -> w_out [128,784], w_in [128,98]
            with tc.tile_pool(name="wpool", bufs=1) as wp:
                w_out = wp.tile([128, cfg.TW_SRC], dt.float32, tag="w_out")
                w_in = wp.tile([128, cfg.TW_DST], dt.float32, tag="w_in")

                def histogram(stream_ap, sc, tw, out_tile, hp, sp):
                    v = hp.tile([128, sc], dt.float16, tag="hv")
                    nc.sync.dma_start(v[:], stream_ap)
                    vs = hp.tile([128, sc], dt.float16, tag="hvs")
                    nc.vector.tensor_copy(vs[:, 0 : sc - 1], v[:, 1:sc])
                    nc.vector.memset(vs[:, sc - 1 : sc], 30000.0)
                    m = hp.tile([128, sc], dt.float16, tag="hm")
                    nc.vector.tensor_tensor(m[:], v[:], vs[:], op=alu.not_equal)
                    idxf = hp.tile([128, sc], dt.float16, tag="hidxf")
                    nc.vector.scalar_tensor_tensor(idxf[:], v[:], 1.0, m[:],
                                                   op0=alu.add, op1=alu.mult)
                    idx16 = hp.tile([128, sc], dt.int16, tag="hidx16")
                    nc.vector.tensor_scalar_add(idx16[:], idxf[:], -1.0)
                    pos16 = hp.tile([128, sc], dt.int16, tag="hpos16")
                    nc.gpsimd.iota(pos16[:], pattern=[[1, sc]], base=1,
                                   channel_multiplier=0)
                    lp16 = sp.tile([128, tw], dt.int16, tag="hlp16")
                    nc.gpsimd.local_scatter(lp16[:], pos16[:], idx16[:],
                                            channels=128, num_elems=tw,
                                            num_idxs=sc)
                    lpf = sp.tile([128, tw], dt.float32, tag="hlpf")
                    nc.vector.tensor_copy(lpf[:], lp16[:])
                    lps = sp.tile([128, tw], dt.float32, tag="hlps")
                    nc.vector.tensor_tensor_scan(lps[:], lpf[:], lpf[:], 0.0,
                                                 op0=alu.max, op1=alu.max)
                    deg = sp.tile([128, tw], dt.float32, tag="hdeg")
                    nc.vector.tensor_copy(deg[:, 0:1], lps[:, 0:1])
                    nc.vector.tensor_sub(deg[:, 1:tw], lps[:, 1:tw],
                                         lps[:, 0 : tw - 1])
                    degc = sp.tile([128, tw], dt.float32, tag="hdegc")
                    nc.vector.tensor_scalar_max(degc[:], deg[:], 1.0)
                    sq = sp.tile([128, tw], dt.float32, tag="hsq")
                    nc.scalar.sqrt(sq[:], degc[:])
                    nc.vector.reciprocal(out_tile[:], sq[:])

                with (tc.tile_pool(name="hist", bufs=1) as hp,
                      tc.tile_pool(name="hsmall", bufs=1) as sp):
                    histogram(shist_t.ap(), cfg.SC_SRC, cfg.TW_SRC, w_out,
                              hp, sp)
                    histogram(dhist_t.ap(), cfg.SC_DST, cfg.TW_DST, w_in,
                              hp, sp)

                # -------- scale pass: scaled[g, f*4+q] = node * w_out[4g+q]
                with tc.tile_pool(name="scale", bufs=3) as scp:
                    RS = 28  # group-rows per partition per tile
                    C4 = cfg.D * 4
                    for j in range(0, cfg.NG, 128 * RS):
                        s = j // 128
                        nt = scp.tile([128, RS, C4], dt.bfloat16, tag="nt")
                        nc.sync.dma_start(
                            nt[:],
                            node_t.ap()[j : j + 128 * RS, :].rearrange(
                                "(r p) c -> p r c", p=128),
                        )
                        st_ = scp.tile([128, RS, C4], dt.bfloat16, tag="st")
                        nc.vector.tensor_mul(
                            st_[:].rearrange("p r (f q) -> p r f q", q=4),
                            nt[:].rearrange("p r (f q) -> p r f q", q=4),
                            w_out[:, s * 4 : (s + RS) * 4]
                            .rearrange("p (r q) -> p r q", q=4)
                            .unsqueeze(2)
                            .broadcast_to((128, RS, cfg.D, 4)),
                        )
                        nc.sync.dma_start(
                            scaled[j : j + 128 * RS, :].rearrange(
                                "(r p) c -> p r c", p=128),
                            st_[:],
                        )

                # ---------------- main loop
                grp_ap = scaled[:, :]
                with (tc.tile_pool(name="idx", bufs=6) as ip,
                      tc.tile_pool(name="gat", bufs=8) as gp,
                      tc.tile_pool(name="meta", bufs=4) as mp,
                      tc.tile_pool(name="blend", bufs=3) as bp,
                      tc.tile_pool(name="sel", bufs=4) as selp,
                      tc.tile_pool(name="mask", bufs=4) as mkp,
                      tc.tile_pool(name="psum", bufs=4, space="PSUM") as pp,
                      tc.tile_pool(name="outp", bufs=4) as op_):
                    iota16 = wp.tile([128, 128], dt.int16, tag="iota16")
                    nc.gpsimd.iota(iota16[:], pattern=[[1, 128]], base=0,
                                   channel_multiplier=0)
                    iotab = wp.tile([128, 128], dt.bfloat16, tag="iotab")
                    nc.vector.tensor_copy(iotab[:], iota16[:])

                    psum_cur = None
                    for c in range(cfg.NCH):
                        it = ip.tile([128, cfg.CH // 16], dt.int16, tag="it")
                        nc.sync.dma_start(it[:], gidx_t.ap()[c])
                        g4 = gp.tile([128, 8, 256], dt.bfloat16, tag="g4")
                        nc.gpsimd.dma_gather(g4[:], grp_ap, it[:],
                                             num_idxs=cfg.CH,
                                             num_idxs_reg=cfg.CH,
                                             elem_size=256,
                                             queue_num=c % 4)
                        dv = mp.tile([128, 8], dt.bfloat16, tag="dv")
                        nc.sync.dma_start(dv[:], dstv_t.ap()[c])
                        qi = mp.tile([128, 8, 4], dt.bfloat16, tag="qi")
                        nc.sync.dma_start(qi[:], qind_t.ap()[c])

                        # blend: tmp[e,k,f,q] = g4[e,k,f*4+q] * qi[e,k,q]
                        tmp = bp.tile([128, 8, cfg.D, 4], dt.bfloat16,
                                      tag="tmp")
                        g4v = g4[:].rearrange("p k (f q) -> p k f q", q=4)
                        nc.vector.tensor_mul(
                            tmp[:], g4v,
                            qi[:].unsqueeze(2).broadcast_to((128, 8, cfg.D, 4)),
                        )
                        # pairwise adds instead of a 4-way reduce (cheaper
                        # on DVE; exact — at most one q lane is nonzero)
                        tv = tmp[:].rearrange("p k f (a b) -> p k f a b", a=2)
                        pr = bp.tile([128, 8, cfg.D, 2], dt.bfloat16,
                                     tag="pr")
                        nc.vector.tensor_add(pr[:], tv[:, :, :, :, 0],
                                             tv[:, :, :, :, 1])
                        gsel = selp.tile([128, 8, cfg.D], dt.bfloat16,
                                         tag="gsel")
                        nc.vector.tensor_add(gsel[:], pr[:, :, :, 0],
                                             pr[:, :, :, 1])

                        mask = mkp.tile([128, 8, 128], dt.bfloat16, tag="mask")
                        nc.vector.tensor_tensor(
                            mask[:],
                            dv[:].unsqueeze(2).broadcast_to((128, 8, 128)),
                            iotab[:].unsqueeze(1).broadcast_to((128, 8, 128)),
                            op=alu.is_equal,
                        )

                        for k in range(8):
                            t = c * 8 + k
                            if t >= cfg.W * cfg.TPW:
                                break
                            w = t // cfg.TPW
                            r = t % cfg.TPW
                            if r == 0:
                                psum_cur = pp.tile([128, cfg.D], dt.float32,
                                                   tag="ps")
                            nc.tensor.matmul(psum_cur[:], lhsT=mask[:, k, :],
                                             rhs=gsel[:, k, :],
                                             start=(r == 0),
                                             stop=(r == cfg.TPW - 1))
                            if r == cfg.TPW - 1:
                                ob = op_.tile([128, cfg.D], dt.float32,
                                              tag="ob")
                                nc.scalar.mul(ob[:], psum_cur[:],
                                              w_in[:, w : w + 1])
                                nc.sync.dma_start(out_t.ap()[w], ob[:])

    nc.compile()
    return nc


# ----------------------------------------------------------------- runner ---
_CACHE = {}


def kernel(u_f, v_f, src, dst, trace=False):
    from concourse import bass_utils

    cfg = CFG
    u_f, v_f = np.asarray(u_f), np.asarray(v_f)
    src, dst = np.asarray(src), np.asarray(dst)

    if "nc" not in _CACHE:
        _CACHE["nc"] = build(cfg)
    nc = _CACHE["nc"]
    ins = host_prep(cfg, u_f, v_f, src, dst)
    res = bass_utils.run_bass_kernel_spmd(
        nc, ins, core_ids=list(range(cfg.NC)), trace=trace
    )
    out = np.concatenate(
        [res.results[k]["rst"].reshape(cfg.SLICE, cfg.D)
         for k in range(cfg.NC)], axis=0
    )
    kernel.last_results = (res,)
    return out[: cfg.N]


# revision 8
# speedup vs baseline: 1.1336x; 1.1336x over previous
"""GCN layer (degree-normalized copy-src/sum) on 8 TRN2 NeuronCores — v2.

Design (single launch per core, SPMD over 8 cores):
  - node table bf16 [100352, 64]; viewed as [25088, 256] = 4-node groups of
    512B so SWDGE dma_gather int16 indices cover the full table (src//4)
    with no buckets and no <512B DMA latency penalty.
  - out-degree + in-degree histograms on device (sorted-residual run-length
    via GPSIMD local_scatter, f16 streams).
  - scale pass: scaled = node * rsqrt(clip(out_deg,1)) (bf16, DRAM scratch).
  - edges sharded by dst slice owner; per dst window of 128 rows, edges
    padded to 18 tiles of 128 slots; gather chunks of 1024 slots.
  - per chunk: DVE blend selects each edge's true row out of its 4-node
    group (quarter one-hot), DVE builds dst one-hot masks, PE matmuls
    aggregate into PSUM per window (f32 accumulation — no scatter at all).
  - psum * rsqrt(clip(in_deg,1)) -> output slice.
"""

import sys

sys.path.insert(0, "/opt/trn_rl_repo")

import numpy as np
import ml_dtypes


class CFG:
    N = 100000
    D = 64
    NC = 8
    SLICE = 12544          # dst rows per core
    W = 98                 # dst windows of 128 per core
    TPW = 18               # tiles (128 slots) per window
    SLOTS = 98 * 18 * 128  # 225792
    CH = 1024              # gather slots per instruction
    NCH = (SLOTS + CH - 1) // CH  # 221 (tail 512 slots unused)
    NPAD = 100352
    NG = 100352 // 4       # 4-node groups
    SC_SRC = 13312         # src hist stream cols (full graph, all cores)
    SC_DST = 2048          # dst hist stream cols (slice local)
    TW_SRC = 784           # nodes per partition (full)
    TW_DST = 98            # dst rows per partition (slice)


# ------------------------------------------------------------- host prep ---
def _wrap16(a):
    """[1024] int16 -> [128, 64] wrapped in 16 partitions, replicated."""
    w = a.reshape(64, 16).T.astype(np.int16)
    return np.tile(w, (8, 1))


def _hist_stream_pr(p, r, sc):
    """Explicit (partition, residual) pairs -> [128, sc] f16 sorted-residual
    stream padded with -1."""
    order = np.lexsort((r, p))
    po, ro = p[order], r[order]
    cnts = np.bincount(po, minlength=128)
    if cnts.max() > sc:
        raise ValueError(f"hist stream overflow: {cnts.max()} > {sc}")
    st = np.full((128, sc), -1.0, np.float16)
    starts = np.concatenate([[0], np.cumsum(cnts)[:-1]])
    pos = np.arange(len(p)) - starts[po]
    st[po, pos] = ro.astype(np.float16)
    return st


def _hist_stream(vals, tw, sc):
    """Values v in [0, 128*tw): partition v%128, residual v//128."""
    return _hist_stream_pr(vals % 128, vals // 128, sc)


def host_prep(cfg, u_f, v_f, src, dst):
    node = np.zeros((cfg.NPAD, cfg.D), np.float32)
    node[: u_f.shape[0]] = u_f
    node[u_f.shape[0] : u_f.shape[0] + v_f.shape[0]] = v_f
    # interleaved 4-node groups: node_iv[g, f*4+q] = node[4g+q, f]
    node_b = np.ascontiguousarray(
        node.astype(ml_dtypes.bfloat16)
        .reshape(cfg.NG, 4, cfg.D)
        .transpose(0, 2, 1)
        .reshape(cfg.NG, cfg.D * 4)
    )

    src = np.asarray(src, dtype=np.int64)
    dst = np.asarray(dst, dtype=np.int64)

    # src hist in interleaved layout: node n -> partition (n//4)%128,
    # residual ((n//4)//128)*4 + n%4  (so w_out[:, G*4+q] matches the
    # scale pass tiling over group rows G)
    g_all = src // 4
    shist = _hist_stream_pr(g_all % 128, (g_all // 128) * 4 + src % 4,
                            cfg.SC_SRC)

    ins = []
    for k in range(cfg.NC):
        m = (dst // cfg.SLICE) == k
        es = src[m]
        ed = dst[m] - k * cfg.SLICE
        dhist = _hist_stream(ed, cfg.TW_DST, cfg.SC_DST)

        order = np.argsort(ed, kind="stable")
        es, ed = es[order], ed[order]
        w = ed // 128
        wcnt = np.bincount(w, minlength=cfg.W)
        if wcnt.max() > cfg.TPW * 128:
            raise ValueError(f"window overflow: {wcnt.max()}")
        # slot for edge i: w*TPW*128 + rank within window
        starts = np.concatenate([[0], np.cumsum(wcnt)[:-1]])
        rank = np.arange(len(ed)) - starts[w]
        slot = w * cfg.TPW * 128 + rank

        tot = cfg.NCH * cfg.CH
        gidx = np.zeros(tot, np.int16)          # pad -> group 0
        dstv = np.full(tot, -1.0, ml_dtypes.bfloat16)
        qind = np.zeros((tot, 4), ml_dtypes.bfloat16)
        gidx[slot] = (es // 4).astype(np.int16)
        dstv[slot] = (ed % 128).astype(np.float32).astype(ml_dtypes.bfloat16)
        qind[slot, es % 4] = 1.0

        gidx_w = np.stack([_wrap16(gidx[c * cfg.CH : (c + 1) * cfg.CH])
                           for c in range(cfg.NCH)])          # [NCH,128,64]
        # slot s: chunk c = s//1024, tile k=(s%1024)//128, partition e=s%128
        dstv_w = dstv.reshape(cfg.NCH, 8, 128).transpose(0, 2, 1)
        qind_w = qind.reshape(cfg.NCH, 8, 128, 4).transpose(0, 2, 1, 3)
        ins.append({
            "node_tbl": node_b, "shist": shist, "dhist": dhist,
            "gidx": gidx_w, "dstv": np.ascontiguousarray(dstv_w),
            "qind": np.ascontiguousarray(qind_w),
        })
    return ins


# ----------------------------------------------------------- device build ---
def build(cfg):
    import concourse.tile as tile
    from concourse import bacc, mybir

    dt = mybir.dt
    alu = mybir.AluOpType

    nc = bacc.Bacc("TRN2", target_bir_lowering=False, debug=False,
                   num_devices=cfg.NC, num_swdge_queues=4)
    node_t = nc.dram_tensor("node_tbl", [cfg.NG, cfg.D * 4], dt.bfloat16,
                            kind="ExternalInput")
    shist_t = nc.dram_tensor("shist", [128, cfg.SC_SRC], dt.float16,
                             kind="ExternalInput")
    dhist_t = nc.dram_tensor("dhist", [128, cfg.SC_DST], dt.float16,
                             kind="ExternalInput")
    gidx_t = nc.dram_tensor("gidx", [cfg.NCH, 128, cfg.CH // 16], dt.int16,
                            kind="ExternalInput")
    dstv_t = nc.dram_tensor("dstv", [cfg.NCH, 128, 8], dt.bfloat16,
                            kind="ExternalInput")
    qind_t = nc.dram_tensor("qind", [cfg.NCH, 128, 8, 4], dt.bfloat16,
                            kind="ExternalInput")
    out_t = nc.dram_tensor("rst", [cfg.W, 128, cfg.D], dt.float32,
                           kind="ExternalOutput")

    with tile.TileContext(nc) as tc:
        with tc.tile_pool(name="dram", bufs=1, space="DRAM") as dpool:
            scaled = dpool.tile([cfg.NG, cfg.D * 4], dt.bfloat16)

            # ---------------- histograms -> w_out [128,784], w_in [128,98]
            with tc.tile_pool(name="wpool", bufs=1) as wp:
                w_out = wp.tile([128, cfg.TW_SRC], dt.float32, tag="w_out")
                w_in = wp.tile([128, cfg.TW_DST], dt.float32, tag="w_in")

                def histogram(stream_ap, sc, tw, out_tile, hp, sp):
                    v = hp.tile([128, sc], dt.float16, tag="hv")
                    nc.sync.dma_start(v[:], stream_ap)
                    vs = hp.tile([128, sc], dt.float16, tag="hvs")
                    nc.vector.tensor_copy(vs[:, 0 : sc - 1], v[:, 1:sc])
                    nc.vector.memset(vs[:, sc - 1 : sc], 30000.0)
                    m = hp.tile([128, sc], dt.float16, tag="hm")
                    nc.vector.tensor_tensor(m[:], v[:], vs[:], op=alu.not_equal)
                    idxf = hp.tile([128, sc], dt.float16, tag="hidxf")
                    nc.vector.scalar_tensor_tensor(idxf[:], v[:], 1.0, m[:],
                                                   op0=alu.add, op1=alu.mult)
                    idx16 = hp.tile([128, sc], dt.int16, tag="hidx16")
                    nc.vector.tensor_scalar_add(idx16[:], idxf[:], -1.0)
                    pos16 = hp.tile([128, sc], dt.int16, tag="hpos16")
                    nc.gpsimd.iota(pos16[:], pattern=[[1, sc]], base=1,
                                   channel_multiplier=0)
                    lp16 = sp.tile([128, tw], dt.int16, tag="hlp16")
                    nc.gpsimd.local_scatter(lp16[:], pos16[:], idx16[:],
                                            channels=128, num_elems=tw,
                                            num_idxs=sc)
                    lpf = sp.tile([128, tw], dt.float32, tag="hlpf")
                    nc.vector.tensor_copy(lpf[:], lp16[:])
                    lps = sp.tile([128, tw], dt.float32, tag="hlps")
                    nc.vector.tensor_tensor_scan(lps[:], lpf[:], lpf[:], 0.0,
                                                 op0=alu.max, op1=alu.max)
                    deg = sp.tile([128, tw], dt.float32, tag="hdeg")
                    nc.vector.tensor_copy(deg[:, 0:1], lps[:, 0:1])
                    nc.vector.tensor_sub(deg[:, 1:tw], lps[:, 1:tw],
                                         lps[:, 0 : tw - 1])
                    degc = sp.tile([128, tw], dt.float32, tag="hdegc")
                    nc.vector.tensor_scalar_max(degc[:], deg[:], 1.0)
                    sq = sp.tile([128, tw], dt.float32, tag="hsq")
                    nc.scalar.sqrt(sq[:], degc[:])
                    nc.vector.reciprocal(out_tile[:], sq[:])

                with (tc.tile_pool(name="hist", bufs=1) as hp,
                      tc.tile_pool(name="hsmall", bufs=1) as sp):
                    histogram(shist_t.ap(), cfg.SC_SRC, cfg.TW_SRC, w_out,
                              hp, sp)
                    histogram(dhist_t.ap(), cfg.SC_DST, cfg.TW_DST, w_in,
                              hp, sp)

                # -------- scale pass: scaled[g, f*4+q] = node * w_out[4g+q]
                with tc.tile_pool(name="scale", bufs=3) as scp:
                    RS = 28  # group-rows per partition per tile
                    C4 = cfg.D * 4
                    for j in range(0, cfg.NG, 128 * RS):
                        s = j // 128
                        nt = scp.tile([128, RS, C4], dt.bfloat16, tag="nt")
                        nc.sync.dma_start(
                            nt[:],
                            node_t.ap()[j : j + 128 * RS, :].rearrange(
                                "(r p) c -> p r c", p=128),
                        )
                        st_ = scp.tile([128, RS, C4], dt.bfloat16, tag="st")
                        nc.vector.tensor_mul(
                            st_[:].rearrange("p r (f q) -> p r f q", q=4),
                            nt[:].rearrange("p r (f q) -> p r f q", q=4),
                            w_out[:, s * 4 : (s + RS) * 4]
                            .rearrange("p (r q) -> p r q", q=4)
                            .unsqueeze(2)
                            .broadcast_to((128, RS, cfg.D, 4)),
                        )
                        nc.sync.dma_start(
                            scaled[j : j + 128 * RS, :].rearrange(
                                "(r p) c -> p r c", p=128),
                            st_[:],
                        )

                # ---------------- main loop
                grp_ap = scaled[:, :]
                with (tc.tile_pool(name="idx", bufs=6) as ip,
                      tc.tile_pool(name="gat", bufs=8) as gp,
                      tc.tile_pool(name="meta", bufs=4) as mp,
                      tc.tile_pool(name="blend", bufs=3) as bp,
                      tc.tile_pool(name="sel", bufs=4) as selp,
                      tc.tile_pool(name="mask", bufs=4) as mkp,
                      tc.tile_pool(name="psum", bufs=4, space="PSUM") as pp,
                      tc.tile_pool(name="outp", bufs=4) as op_):
                    iota16 = wp.tile([128, 128], dt.int16, tag="iota16")
                    nc.gpsimd.iota(iota16[:], pattern=[[1, 128]], base=0,
                                   channel_multiplier=0)
                    iotab = wp.tile([128, 128], dt.bfloat16, tag="iotab")
                    nc.vector.tensor_copy(iotab[:], iota16[:])

                    psum_cur = None
                    for c in range(cfg.NCH):
                        it = ip.tile([128, cfg.CH // 16], dt.int16, tag="it")
                        nc.sync.dma_start(it[:], gidx_t.ap()[c])
                        g4 = gp.tile([128, 8, 256], dt.bfloat16, tag="g4")
                        nc.gpsimd.dma_gather(g4[:], grp_ap, it[:],
                                             num_idxs=cfg.CH,
                                             num_idxs_reg=cfg.CH,
                                             elem_size=256,
                                             queue_num=c % 4)
                        dv = mp.tile([128, 8], dt.bfloat16, tag="dv")
                        nc.sync.dma_start(dv[:], dstv_t.ap()[c])
                        qi = mp.tile([128, 8, 4], dt.bfloat16, tag="qi")
                        nc.sync.dma_start(qi[:], qind_t.ap()[c])

                        # blend: tmp[e,k,f,q] = g4[e,k,f*4+q] * qi[e,k,q]
                        tmp = bp.tile([128, 8, cfg.D, 4], dt.bfloat16,
                                      tag="tmp")
                        g4v = g4[:].rearrange("p k (f q) -> p k f q", q=4)
                        nc.vector.tensor_mul(
                            tmp[:], g4v,
                            qi[:].unsqueeze(2).broadcast_to((128, 8, cfg.D, 4)),
                        )
                        gsel = selp.tile([128, 8, cfg.D], dt.bfloat16,
                                         tag="gsel")
                        with nc.allow_low_precision(
                                reason="4-way one-hot select: <=1 nonzero"):
                            nc.vector.reduce_sum(gsel[:], tmp[:],
                                                 axis=mybir.AxisListType.X)

                        mask = mkp.tile([128, 8, 128], dt.bfloat16, tag="mask")
                        nc.vector.tensor_tensor(
                            mask[:],
                            dv[:].unsqueeze(2).broadcast_to((128, 8, 128)),
                            iotab[:].unsqueeze(1).broadcast_to((128, 8, 128)),
                            op=alu.is_equal,
                        )

                        for k in range(8):
                            t = c * 8 + k
                            if t >= cfg.W * cfg.TPW:
                                break
                            w = t // cfg.TPW
                            r = t % cfg.TPW
                            if r == 0:
                                psum_cur = pp.tile([128, cfg.D], dt.float32,
                                                   tag="ps")
                            nc.tensor.matmul(psum_cur[:], lhsT=mask[:, k, :],
                                             rhs=gsel[:, k, :],
                                             start=(r == 0),
                                             stop=(r == cfg.TPW - 1))
                            if r == cfg.TPW - 1:
                                ob = op_.tile([128, cfg.D], dt.float32,
                                              tag="ob")
                                nc.scalar.mul(ob[:], psum_cur[:],
                                              w_in[:, w : w + 1])
                                nc.sync.dma_start(out_t.ap()[w], ob[:])

    nc.compile()
    return nc


# ----------------------------------------------------------------- runner ---
_CACHE = {}


def kernel(u_f, v_f, src, dst, trace=False):
    from concourse import bass_utils

    cfg = CFG
    u_f, v_f = np.asarray(u_f), np.asarray(v_f)
    src, dst = np.asarray(src), np.asarray(dst)

    if "nc" not in _CACHE:
        _CACHE["nc"] = build(cfg)
    nc = _CACHE["nc"]
    ins = host_prep(cfg, u_f, v_f, src, dst)
    res = bass_utils.run_bass_kernel_spmd(
        nc, ins, core_ids=list(range(cfg.NC)), trace=trace
    )
    out = np.concatenate(
        [res.results[k]["rst"].reshape(cfg.SLICE, cfg.D)
         for k in range(cfg.NC)], axis=0
    )
    kernel.last_results = (res,)
    return out[: cfg.N]
